# revision 14
# baseline (speedup 1.0000x reference)
"""Trainium2 Bass kernel for the 4-layer dendritic-LIF SNN.

Strategy: data-parallel over batch (128 -> 16 per core, 8 cores, no
collectives).  Within a core, all layer matmuls are batched over the full
(T=100) x (Bc=16) row set — only the elementwise LIF state updates are
sequential in time.

v2 optimizations (validated against the instruction cost model / TimelineSim):
 - all big matmuls run with bf16 operands (spikes are exactly 0/1 in bf16;
   x/weight rounding averages out over >=512-term dot products): fp32-rhs
   matmuls cost 4 cycles/row vs 1 for bf16.
 - the dendrite bias is folded away via e[t] = d[t] - b (bias-free
   recurrence with scan initial = -b), and (1-beta) is folded into the
   weights, so the matmul PSUM is directly the scan's data1 operand — the
   640 per-(m,bl) Activation evicts of v1 are gone.  The branch sum gets the
   bias back via the (1-alpha)-scaled eviction's bias AP.
 - branch-sum over K=4 runs as PSUM accumulation of identity matmuls with
   f32r rhs of 400 columns (1 cycle/row; fp32 exactness).
 - weights/x are pre-laid-out on host so each (ob,k4) weight block is ONE
   large-line DMA: HWDGE charges ~630ns per DMA instruction, serialized —
   v1's ~1030 tile-sized DMAs were the top device (650us); v2 issues ~60.

Toolchain workarounds (empirically validated):
 - instructions may carry at most 1 sem-wait -> split extras onto NOPs
 - tensor_tensor_scan `initial` must be an AP
"""
import os
import sys
import time

import numpy as np
import ml_dtypes

for _p in ("/root/.axon_site/_ro/trn_rl_repo", "/opt/trn_rl_repo"):
    if os.path.isdir(_p) and _p not in sys.path:
        sys.path.append(_p)

import concourse.bass as bass
import concourse.mybir as mybir
import concourse.tile as tile_mod
from concourse.tile import TileContext
from concourse.vector_clock import ScopedClock

f32 = mybir.dt.float32
bf16 = mybir.dt.bfloat16
f32r = mybir.dt.float32r
AL = mybir.AluOpType
AF = mybir.ActivationFunctionType

# ---------------------------------------------------------------- problem dims
B, T, IN, K = 128, 100, 2752, 4
INP = 2816              # IN padded to 22*128
H1, H2, H3, NCLS = 512, 512, 256, 100
NCORES = 8
BC = B // NCORES        # 16 samples per core
HALF = BC // 2          # 8 samples per half-pass

# ------------------------------------------------------- tile workarounds
_MAX_WAITS = 1

_orig_lower = tile_mod.TileContext._lower_ordered_insts


def _split_waits_in_dict(nc, ordered):
    for bb_name, insts in ordered.items():
        new_list = []
        changed = False
        for inst in insts:
            si = inst.sync_info
            if si is not None and len(si.on_wait) > _MAX_WAITS:
                changed = True
                waits = list(si.on_wait)
                keep, extra = waits[:_MAX_WAITS], waits[_MAX_WAITS:]
                for w in extra:
                    nop = mybir.InstNoOp(
                        name=nc.get_next_instruction_name(), ins=[], outs=[]
                    )
                    nop.engine = inst.engine
                    nop.sync_info = mybir.SyncInfo(on_wait=[w], on_update=[])
                    nc.register_instruction(nop, overwrite=True)
                    new_list.append(nop)
                inst.sync_info = mybir.SyncInfo(
                    on_wait=keep, on_update=list(si.on_update)
                )
            new_list.append(inst)
        if changed:
            insts[:] = new_list


def _patched_lower(self, ordered):
    _split_waits_in_dict(self.nc, ordered)
    return _orig_lower(self, ordered)


def _patched_drain_and_barrier(self, tick_clock, wait_clock):
    drain_inst = self.nc.sync.drain()
    wait_clock.add_sem_waits(
        drain_inst.ins, ScopedClock({None: tick_clock.global_clock})
    )
    si = drain_inst.ins.sync_info
    if si is not None and len(si.on_wait) > 1:
        waits = list(si.on_wait)
        drain_inst.ins.sync_info = mybir.SyncInfo(
            on_wait=[waits[0]], on_update=list(si.on_update)
        )
        for w in waits[1:]:
            n2 = self.nc.sync.nop()
            n2.ins.sync_info = mybir.SyncInfo(on_wait=[w], on_update=[])
    self.nc.all_engine_barrier()
    popped = self.nc._tile_sem_poison_stack.pop()
    assert popped is self._sem_poison
    self.nc.clear_and_free_semaphores(list(self.sems.allocated().values()))
    self.nc.all_engine_barrier()


tile_mod.TileContext._lower_ordered_insts = _patched_lower
tile_mod.TileContext._drain_and_barrier = _patched_drain_and_barrier

# tuning knobs (sim-guided A/B; both regressed in TimelineSim -> off)
_POOL_SCANS = os.environ.get("K_POOL_SCANS", "0") == "1"
_POOL_MEM = os.environ.get("K_POOL_MEM", "0") == "1"

# ------------------------------------------------------------- const layout
# ctab column offsets: per-layer scan initials (-b), (1-alpha), bias
# (1-alpha)*sum_k b, alpha broadcast, mem0
_N_M = (K * H1 // 128, K * H2 // 128, K * H3 // 128)      # 16, 16, 8
_N_OB = (H1 // 128, H2 // 128, H3 // 128)                 # 4, 4, 2

_CT = {}
_off = 0
for _l in range(3):
    _CT[f"binit{_l+1}"] = _off; _off += _N_M[_l]
for _l in range(3):
    _CT[f"oma{_l+1}"] = _off; _off += _N_OB[_l]
for _l in range(3):
    _CT[f"sumb{_l+1}"] = _off; _off += _N_OB[_l]
for _l in range(3):
    _CT[f"altab{_l+1}"] = _off; _off += _N_OB[_l] * BC
for _l in range(3):
    _CT[f"mem0{_l+1}"] = _off; _off += _N_OB[_l] * BC
_CTW = _off  # 380


# ---------------------------------------------------------------- the program
PHASE_MARKS = []


def _mark(nc, label):
    nm = nc.get_next_instruction_name()
    PHASE_MARKS.append((label, int(nm.split('-')[1])))


def _build_program():
    nc = bass.Bass()
    PHASE_MARKS.clear()

    def din(name, shape, dt=f32):
        return nc.dram_tensor(name, shape, dt, kind="ExternalInput")

    xS = din("xS", [128, 22 * BC * T], bf16)       # col k*1600 + b*100 + t
    w1S = din("w1S", [128, _N_M[0] * 22 * 128], bf16)   # col (m*22+k)*128+j
    w2S = din("w2S", [128, _N_M[1] * 4 * 128], bf16)
    w3S = din("w3S", [128, _N_M[2] * 4 * 128], bf16)
    w4T = din("w4T", [H3, NCLS])
    bt1 = din("bt1", [128, _N_M[0] * T])           # beta bcast, col m*100+t
    bt2 = din("bt2", [128, _N_M[1] * T])
    bt3 = din("bt3", [128, _N_M[2] * T])
    ctab_d = din("ctab", [128, _CTW])
    ident = din("ident", [128, 128])
    b4c = din("b4c", [NCLS, 1])
    out = nc.dram_tensor("out", [NCLS, BC], f32, kind="ExternalOutput")

    CT = BC * T          # 1600 cols per contraction tile

    with TileContext(nc) as tc:
        with (
            tc.tile_pool(name="const", bufs=1) as cpool,
            tc.tile_pool(name="spk", bufs=1) as spool,
            tc.tile_pool(name="state", bufs=1) as stpool,
        ):
            ident_sb = cpool.tile([128, 128], f32r)
            nc.sync.dma_start(out=ident_sb[:], in_=ident[:].bitcast(f32r))
            ctab = cpool.tile([128, _CTW], f32)
            nc.sync.dma_start(out=ctab[:], in_=ctab_d[:])
            z64 = cpool.tile([128, 64], f32)
            nc.vector.memset(z64[:], 0.0)

            bts = {}
            for nm, dr, w in (("bt1", bt1, _N_M[0] * T),
                              ("bt2", bt2, _N_M[1] * T),
                              ("bt3", bt3, _N_M[2] * T)):
                t_ = cpool.tile([128, w], f32, tag=nm)
                nc.sync.dma_start(out=t_[:], in_=dr[:])
                bts[nm] = t_

            spk1 = spool.tile([128, K * CT], bf16, tag="spk1")
            spk2 = spool.tile([128, K * CT], bf16, tag="spk2")
            spk3 = spool.tile([128, (H3 // 128) * CT], bf16, tag="spk3")

            mem_t = {}
            for li in (1, 2, 3):
                w = _N_OB[li - 1] * BC
                t_ = stpool.tile([128, w], f32, tag=f"mem{li}")
                off = _CT[f"mem0{li}"]
                nc.vector.tensor_scalar(
                    out=t_[:], in0=ctab[:, off:off + w],
                    scalar1=0.0, scalar2=None, op0=AL.add,
                )
                mem_t[li] = t_

            # ---------------------------------------------------- layer pass
            def layer_pass(li, kt, n_oblk, wS, btsb, ds, rhs_of):
                """Matmul + dendrite scan + branch reduce for one layer.
                li: layer idx, kt: contraction tiles, n_oblk: H/128,
                rhs_of(k, off) -> [128,400] bf16 rhs AP at column offset."""
                o_hi_w = n_oblk * BC
                nm = li - 1
                with (
                    tc.tile_pool(name=f"w{li}", bufs=2) as wpool,
                    tc.tile_pool(name=f"dt{li}", bufs=2) as dtpool,
                    tc.tile_pool(name=f"mm{li}", bufs=2, space="PSUM") as mmps,
                    tc.tile_pool(name=f"dp{li}", bufs=1, space="PSUM") as dpps,
                ):
                    for ob in range(n_oblk):
                        Dp = {}
                        for h in range(2):
                            for g in range(2):
                                Dp[(h, g)] = dpps.tile(
                                    [128, 400], f32,
                                    tag=f"dp{h}{g}", name=f"dp{h}{g}",
                                )
                        for k4 in range(K):
                            m = k4 * n_oblk + ob
                            w_ = wpool.tile([128, kt * 128], bf16, tag="w")
                            nc.sync.dma_start(
                                out=w_[:],
                                in_=wS[:, m * kt * 128:(m + 1) * kt * 128],
                            )
                            for h in range(2):
                                ps = [
                                    mmps.tile([128, 400], f32, tag=f"mm{n}",
                                              name=f"mm{n}")
                                    for n in range(2)
                                ]
                                for k in range(kt):
                                    for n in range(2):
                                        nc.tensor.matmul(
                                            ps[n][:],
                                            w_[:, k * 128:(k + 1) * 128],
                                            rhs_of(k, h * 800 + n * 400),
                                            start=(k == 0),
                                            stop=(k == kt - 1),
                                        )
                                # f32r so walrus accepts it as the f32r
                                # branch-sum matmul's rhs (values are rounded
                                # to f32r at the scan's write)
                                dts = dtpool.tile(
                                    [128, 800], f32r, tag=f"dts{h}"
                                )
                                for bl in range(HALF):
                                    eng = (
                                        nc.gpsimd
                                        if (_POOL_SCANS and bl % 2)
                                        else nc.vector
                                    )
                                    eng.tensor_tensor_scan(
                                        out=dts[:, bl * T:(bl + 1) * T],
                                        data0=btsb[:, m * T:(m + 1) * T],
                                        data1=ps[bl // 4][
                                            :, (bl % 4) * T:(bl % 4 + 1) * T
                                        ],
                                        initial=ctab[
                                            :, _CT[f"binit{li}"] + m:
                                            _CT[f"binit{li}"] + m + 1
                                        ],
                                        op0=AL.mult,
                                        op1=AL.add,
                                    )
                                for g in range(2):
                                    nc.tensor.matmul(
                                        Dp[(h, g)][:],
                                        ident_sb[:],
                                        dts[:, g * 400:(g + 1) * 400],
                                        start=(k4 == 0),
                                        stop=(k4 == K - 1),
                                    )
                        # evict branch-summed D into ds with (1-alpha) and
                        # the bias correction (1-alpha)*sum_k b
                        for h in range(2):
                            for g in range(2):
                                off = ob * BC + h * HALF + g * 4
                                dst = ds[:].rearrange(
                                    "p (t c) -> p c t", c=o_hi_w
                                )
                                nc.scalar.activation(
                                    dst[:, off:off + 4, :],
                                    Dp[(h, g)][:].rearrange(
                                        "p (b t) -> p b t", b=4
                                    ),
                                    AF.Identity,
                                    bias=ctab[
                                        :, _CT[f"sumb{li}"] + ob:
                                        _CT[f"sumb{li}"] + ob + 1
                                    ],
                                    scale=ctab[
                                        :, _CT[f"oma{li}"] + ob:
                                        _CT[f"oma{li}"] + ob + 1
                                    ],
                                )

            # ----------------------------------------------------- mem scan
            def mem_scan(li, n_oblk, ds, mem, spk):
                o_hi_w = n_oblk * BC
                al0 = _CT[f"altab{li}"]
                altab = ctab[:, al0:al0 + o_hi_w]
                with tc.tile_pool(name=f"ms{li}", bufs=3) as msp:
                    spk_r = spk[:].rearrange(
                        "p (o b t) -> p o b t", o=n_oblk, b=BC
                    )
                    for t in range(T):
                        ds_t = ds[:, t * o_hi_w:(t + 1) * o_hi_w].rearrange(
                            "p (o b) -> p o b", o=n_oblk
                        )
                        if t == 0:
                            prev = z64[:, :o_hi_w].rearrange(
                                "p (o b) -> p o b", o=n_oblk
                            )
                        else:
                            prev = spk_r[:, :, :, t - 1]
                        u = msp.tile([128, o_hi_w], f32, tag="u")
                        nc.vector.tensor_tensor(
                            out=u[:].rearrange("p (o b) -> p o b", o=n_oblk),
                            in0=ds_t,
                            in1=prev,
                            op=AL.subtract,
                        )
                        # the alpha-multiply can run on the (otherwise idle)
                        # GpSimd engine: the DVE serial chain per timestep
                        # drops from 4 ops to 3
                        v = msp.tile([128, o_hi_w], f32, tag="v")
                        veng = nc.gpsimd if _POOL_MEM else nc.vector
                        veng.tensor_tensor(
                            out=v[:], in0=mem[:], in1=altab, op=AL.mult
                        )
                        nc.vector.tensor_tensor(
                            out=mem[:], in0=v[:], in1=u[:], op=AL.add
                        )
                        nc.vector.tensor_scalar(
                            out=spk_r[:, :, :, t],
                            in0=mem[:].rearrange("p (o b) -> p o b", o=n_oblk),
                            scalar1=1.0,
                            scalar2=None,
                            op0=AL.is_gt,
                        )

            # -------------------------------------------------------- layer 1
            _mark(nc, "L1")
            with tc.tile_pool(name="xp", bufs=1) as xpool:
                xsb = xpool.tile([128, 22 * CT], bf16)
                for c in range(4):
                    c0 = c * (22 * CT // 4)
                    c1 = (c + 1) * (22 * CT // 4)
                    nc.sync.dma_start(out=xsb[:, c0:c1], in_=xS[:, c0:c1])

                with tc.tile_pool(name="ds1p", bufs=1) as ds1p:
                    ds1 = ds1p.tile([128, T * 64], f32)
                    layer_pass(
                        1, 22, 4, w1S, bts["bt1"], ds1,
                        lambda k, off: xsb[:, k * CT + off:k * CT + off + 400],
                    )
                    _mark(nc, "mem1")
                    mem_scan(1, 4, ds1, mem_t[1], spk1)

            # -------------------------------------------------------- layer 2
            _mark(nc, "L2")
            with tc.tile_pool(name="ds2p", bufs=1) as ds2p:
                ds2 = ds2p.tile([128, T * 64], f32)
                layer_pass(
                    2, 4, 4, w2S, bts["bt2"], ds2,
                    lambda k, off: spk1[:, k * CT + off:k * CT + off + 400],
                )
                _mark(nc, "mem2")
                mem_scan(2, 4, ds2, mem_t[2], spk2)

            # -------------------------------------------------------- layer 3
            _mark(nc, "L3")
            with tc.tile_pool(name="ds3p", bufs=1) as ds3p:
                ds3 = ds3p.tile([128, T * 32], f32)
                layer_pass(
                    3, 4, 2, w3S, bts["bt3"], ds3,
                    lambda k, off: spk2[:, k * CT + off:k * CT + off + 400],
                )
                _mark(nc, "mem3")
                mem_scan(3, 2, ds3, mem_t[3], spk3)

            # -------------------------------------------------------- layer 4
            _mark(nc, "L4")
            with (
                tc.tile_pool(name="l4", bufs=1) as l4p,
                tc.tile_pool(name="l4ps", bufs=1, space="PSUM") as l4ps,
            ):
                ps4 = l4ps.tile([NCLS, BC], f32)
                for kk in range(H3 // 128):
                    red = l4p.tile([128, BC], f32, tag=f"red{kk}")
                    nc.vector.tensor_reduce(
                        out=red[:],
                        in_=spk3[
                            :, kk * CT:(kk + 1) * CT
                        ].rearrange("p (b t) -> p b t", b=BC),
                        axis=mybir.AxisListType.X,
                        op=AL.add,
                    )
                    w4_ = l4p.tile([128, NCLS], f32, tag=f"w4{kk}")
                    nc.sync.dma_start(
                        out=w4_[:], in_=w4T[kk * 128:(kk + 1) * 128, :]
                    )
                    nc.tensor.matmul(
                        ps4[:], w4_[:], red[:],
                        start=(kk == 0), stop=(kk == H3 // 128 - 1),
                    )
                b4sb = l4p.tile([NCLS, 1], f32)
                nc.sync.dma_start(out=b4sb[:], in_=b4c[:])
                osb = l4p.tile([NCLS, BC], f32)
                nc.scalar.activation(
                    osb[:], ps4[:], AF.Identity,
                    bias=b4sb[:, 0:1], scale=1.0 / T,
                )
                nc.sync.dma_start(out=out[:], in_=osb[:])

    return nc


_NC_CACHE = None


def _get_program():
    global _NC_CACHE
    if _NC_CACHE is None:
        _NC_CACHE = _build_program()
    return _NC_CACHE


# ---------------------------------------------------------------- host prep
def _sigmoid(x):
    return 1.0 / (1.0 + np.exp(-np.asarray(x, np.float64)))


def _km(a, O):
    """(O*K,...) o-major rows -> k-major rows (K*O, ...)."""
    return a.reshape(O, K, *a.shape[1:]).transpose(1, 0, *range(2, a.ndim + 1)).reshape(K * O, *a.shape[1:])


def _layer_tables(W, b, tau_m, tau_n, mask, O, kt):
    """Returns dict with wS (bf16 SBUF-layout weights, (1-beta) folded in),
    btab (beta bcast), binit (-b), oma (1-alpha), sumb ((1-alpha)*sum_k b),
    altab (alpha bcast)."""
    Wm = (W * mask).astype(np.float64)          # (O*K, In), o-major rows
    Wkm = _km(Wm, O)                            # k-major rows (K*O, In)
    beta = _sigmoid(tau_n).reshape(O, K).T.reshape(-1)      # k-major (K*O,)
    bkm = _km(b.astype(np.float64), O)                      # k-major (K*O,)
    alpha = _sigmoid(tau_m)                     # (O,)
    omb = 1.0 - beta
    n_m = (O * K) // 128
    n_ob = O // 128
    In = Wkm.shape[1]
    Inp = kt * 128
    Ws = Wkm * omb[:, None]                     # fold (1-beta) into weights
    Wp = np.zeros((n_m * 128, Inp), np.float64)
    Wp[:, :In] = Ws
    # wS[p, (m*kt + k)*128 + j] = Wp[m*128+j, k*128+p]
    wS = np.ascontiguousarray(
        Wp.reshape(n_m, 128, kt, 128).transpose(3, 0, 2, 1)
        .reshape(128, n_m * kt * 128)
    ).astype(ml_dtypes.bfloat16)
    btab = np.ascontiguousarray(
        np.repeat(beta.reshape(n_m, 128).T[:, :, None], T, axis=2)
        .reshape(128, n_m * T)
    ).astype(np.float32)
    binit = np.ascontiguousarray(
        (-bkm).reshape(n_m, 128).T
    ).astype(np.float32)                                   # (128, n_m)
    oma = np.ascontiguousarray(
        (1.0 - alpha).reshape(n_ob, 128).T
    ).astype(np.float32)                                   # (128, n_ob)
    sumb_o = bkm.reshape(K, O).sum(axis=0)                 # (O,) sum_k b
    sumb = np.ascontiguousarray(
        ((1.0 - alpha) * sumb_o).reshape(n_ob, 128).T
    ).astype(np.float32)                                   # (128, n_ob)
    altab = np.ascontiguousarray(
        np.repeat(alpha.reshape(n_ob, 128).T[:, :, None], BC, axis=2)
        .reshape(128, n_ob * BC)
    ).astype(np.float32)
    return dict(wS=wS, btab=btab, binit=binit, oma=oma, sumb=sumb,
                altab=altab)


def _mem0_rearrange(m0, O):
    # (BC, O) -> [128, n_ob*BC] with [p, o_hi*BC + b] = m0[b, o_hi*128+p]
    n_ob = O // 128
    return np.ascontiguousarray(
        m0.T.reshape(n_ob, 128, BC).transpose(1, 0, 2).reshape(128, n_ob * BC)
    ).astype(np.float32)


LAST_EXEC_NS = None

_EXEC_CACHE = None


def _get_exec():
    """Build (once) a cached jitted PJRT executable for the Bass program,
    mirroring concourse.bass2jax.run_bass_via_pjrt so repeat calls skip
    walrus compilation and can be timed."""
    global _EXEC_CACHE
    if _EXEC_CACHE is not None:
        return _EXEC_CACHE
    import jax
    from jax.sharding import Mesh, PartitionSpec
    from jax.experimental.shard_map import shard_map
    import concourse.mybir as _mb
    from concourse import bass2jax as b2j

    nc = _get_program()
    b2j.install_neuronx_cc_hook()
    partition_name = (
        nc.partition_id_tensor.name if nc.partition_id_tensor else None
    )
    in_names, out_names, out_avals, zero_outs = [], [], [], []
    for alloc in nc.m.functions[0].allocations:
        if not isinstance(alloc, _mb.MemoryLocationSet):
            continue
        name = alloc.memorylocations[0].name
        if alloc.kind == "ExternalInput":
            if name != partition_name:
                in_names.append(name)
        elif alloc.kind == "ExternalOutput":
            shape = tuple(alloc.tensor_shape)
            dtype = _mb.dt.np(alloc.dtype)
            out_names.append(name)
            out_avals.append(jax.core.ShapedArray(shape, dtype))
            zero_outs.append(np.zeros(shape, dtype))
    n_params = len(in_names)
    all_in_names = list(in_names) + list(out_names)
    if partition_name is not None:
        all_in_names.append(partition_name)
    donate = tuple(range(n_params, n_params + len(out_names)))

    def _body(*args):
        operands = list(args)
        if partition_name is not None:
            operands.append(b2j.partition_id_tensor())
        outs = b2j._bass_exec_p.bind(
            *operands,
            out_avals=tuple(out_avals),
            in_names=tuple(all_in_names),
            out_names=tuple(out_names),
            lowering_input_output_aliases=(),
            sim_require_finite=True,
            sim_require_nnan=True,
            nc=nc,
        )
        return tuple(outs)

    devices = jax.devices()[:NCORES]
    mesh = Mesh(np.asarray(devices), ("core",))
    in_specs = (PartitionSpec("core"),) * (n_params + len(out_names))
    out_specs = (PartitionSpec("core"),) * len(out_names)
    sharded = jax.jit(
        shard_map(
            _body, mesh=mesh, in_specs=in_specs, out_specs=out_specs,
            check_rep=False,
        ),
        donate_argnums=donate,
        keep_unused=True,
    )
    _EXEC_CACHE = (sharded, in_names, out_names, out_avals, zero_outs, mesh)
    return _EXEC_CACHE


def _run_on_device(in_maps, repeats=1):
    """Execute the cached program; returns (per-core outputs, best_exec_ns).
    Inputs are placed on-device once so repeat timings measure execution,
    not host->device transfer of the ~25MB/core weight set."""
    import jax
    from jax.sharding import NamedSharding, PartitionSpec

    sharded, in_names, out_names, out_avals, zero_outs, mesh = _get_exec()
    concat_in = [
        np.concatenate([in_maps[c][n] for c in range(NCORES)], axis=0)
        for n in in_names
    ]
    shd = NamedSharding(mesh, PartitionSpec("core"))
    dev_in = [jax.device_put(a, shd) for a in concat_in]
    for a in dev_in:
        a.block_until_ready()
    best = None
    out_arrs = None
    for _ in range(max(1, repeats)):
        concat_zeros = [
            jax.device_put(
                np.zeros((NCORES * z.shape[0], *z.shape[1:]), z.dtype), shd
            )
            for z in zero_outs
        ]
        for a in concat_zeros:
            a.block_until_ready()
        t0 = time.perf_counter()
        out_arrs = sharded(*dev_in, *concat_zeros)
        out_arrs = [np.asarray(a) for a in out_arrs]
        dt = time.perf_counter() - t0
        if best is None or dt < best:
            best = dt
    results = [
        {
            n: out_arrs[i].reshape(NCORES, *out_avals[i].shape)[c]
            for i, n in enumerate(out_names)
        }
        for c in range(NCORES)
    ]
    return results, int(best * 1e9)


def kernel(
    dvs_inp, W1, b1, tau_m1, tau_n1, mask1,
    W2, b2, tau_m2, tau_n2, mask2,
    W3, b3, tau_m3, tau_n3, mask3,
    W4, b4, mem1_0, mem2_0, mem3_0,
):
    global LAST_EXEC_NS
    _get_program()

    t1 = _layer_tables(W1, b1, tau_m1, tau_n1, mask1, H1, 22)
    t2 = _layer_tables(W2, b2, tau_m2, tau_n2, mask2, H2, 4)
    t3 = _layer_tables(W3, b3, tau_m3, tau_n3, mask3, H3, 4)

    shared = {
        "w1S": t1["wS"], "w2S": t2["wS"], "w3S": t3["wS"],
        "w4T": np.ascontiguousarray(W4.T.astype(np.float32)),
        "bt1": t1["btab"], "bt2": t2["btab"], "bt3": t3["btab"],
        "ident": np.eye(128, dtype=np.float32),
        "b4c": np.ascontiguousarray(b4.astype(np.float32)[:, None]),
    }
    x_all = np.asarray(dvs_inp, np.float32).reshape(B, T, IN)
    in_maps = []
    for c in range(NCORES):
        b0 = c * BC
        # xS[p, k*1600 + b*100 + t] = x_pad[k*128+p, b, t]
        xc = np.zeros((INP, BC, T), np.float32)
        xc[:IN] = x_all[b0:b0 + BC].transpose(2, 0, 1)
        xs = np.ascontiguousarray(
            xc.reshape(22, 128, BC * T).transpose(1, 0, 2)
            .reshape(128, 22 * BC * T)
        ).astype(ml_dtypes.bfloat16)
        ct = np.zeros((128, _CTW), np.float32)
        for li, tt in ((1, t1), (2, t2), (3, t3)):
            for base, key in (("binit", "binit"), ("oma", "oma"),
                              ("sumb", "sumb"), ("altab", "altab")):
                arr = tt[key]
                off = _CT[f"{base}{li}"]
                ct[:, off:off + arr.shape[1]] = arr
        for li, m0, O in ((1, mem1_0, H1), (2, mem2_0, H2), (3, mem3_0, H3)):
            arr = _mem0_rearrange(np.asarray(m0)[b0:b0 + BC], O)
            off = _CT[f"mem0{li}"]
            ct[:, off:off + arr.shape[1]] = arr
        m = dict(shared)
        m["xS"] = xs
        m["ctab"] = ct
        in_maps.append(m)

    results, exec_ns = _run_on_device(
        in_maps, repeats=int(os.environ.get("KERNEL_REPEATS", "1"))
    )
    LAST_EXEC_NS = exec_ns

    out_full = np.empty((B, NCLS), np.float32)
    for c in range(NCORES):
        out_full[c * BC:(c + 1) * BC] = results[c]["out"].T
    return out_full
